# revision 93
# baseline (speedup 1.0000x reference)
"""Trainium2 Bass kernel for nn_C2f_DualModal_MoE (v7).

Full inputs in, full outputs out. Data-parallel over batch: 16 items on
8 cores (2 per core).

Precision plan (rel-err budget 2e-2, measured 1.61e-2 on 8 cores):
  - cv1 as fp8 e4m3 hi/lo DoubleRow: x and 64*w1 are split on the host
    into e4m3 (hi) plus e4m3 of 16x the residual (lo); psum = Ah@xh +
    (Al/16)@xh + (Ah/16)@xl in 3 DR matmuls (600 PE cyc per 400-col
    tile vs bf16's 800), max rel err ~0.13% (better than bf16's 0.26%).
  - expert 3x3 convs in fp8 e4m3 DoubleRow, weights*64: 9 taps = 5 row
    pairs; the 5th pair's second half reads mp8's all-ones front rows
    (negative stride) with 64*bexp in partition 0 of its lhs, so the
    expert bias lands in PSUM for free and the acts are bias-free.
  - cv2 in bf16 (fp8 here measures 3.6e-2 -- over budget); out DMA'd
    as bf16 and upcast on the host.
  - routing exact f32; pooled partials in bf16 (margin is ~16x noise).

Engine budget per core: Act ~84us busy (the span bottleneck: 96 silu
evacuations of 800 cols at 1.2GHz + 185ns access latency each; no
fast mode exists on the Act engine), PE ~76us, DVE/SP/Pool < 37us.

Schedule: p1 (cv1) feeds a p2 software pipeline of per-group expert ->
moe-combine -> cv2 stages; cv2 lags exp by one group. The p1 work is
split into independent a/m HALF-groups: all m halves run before p2
(routing needs pooled(m)), while 14 of the 20 a halves spread through
p2 as Act filler work -- the late p2 cycles are PE-sequence-bound
(exp+cv2 matmuls = 3.7us vs 3.4us of act work per group), so the
fillers are what keeps the Act engine saturated there. PSUM: pse tag
(exp + a-fillers) and pso tag (cv2 + m-fillers + routing), each 2
slots x 2 banks; the slot-WAR rotation is what sequences the pipeline.
Gates broadcast is emitted lazily after the first exp group's matmuls
(any earlier it barriers the in-order PE queue behind the DVE gates
chain). Routing(0) is emitted before item 1's preamble halves to keep
its DVE chain (pooled -> top2 -> expert-weight gathers) unobstructed.
"""

import sys

for _p in ("/opt/trn_rl_repo", "/opt/pypackages"):
    if _p not in sys.path:
        sys.path.append(_p)

import numpy as np
import ml_dtypes
import concourse.bass as bass
import concourse.mybir as mybir
import concourse.tile as tile
from concourse import bacc
from concourse.bass import ds, AP
from concourse.bass_utils import run_bass_kernel_spmd

F32 = mybir.dt.float32
BF16 = mybir.dt.bfloat16
FP8 = mybir.dt.float8e4
AF = mybir.ActivationFunctionType
DR = mybir.MatmulPerfMode.DoubleRow
ALU = mybir.AluOpType

N_CORES = 8
B = 16
BPC = B // N_CORES
C1 = 256
C = 128
E = 4
H = W = 80
S = H * W  # 6400
R = 5  # image rows per spatial tile
N = R * W  # 400 matmul columns per tile
NT = H // R  # 16 tiles
NG = NT // 2  # 8 groups of 2 tiles
GN = 2 * N  # 800 cols per group
HP = H + 2  # 82
HP8 = HP + 1  # 83: fp8 m copy has spare row/col so tap-pair APs stay in-bounds
NV = R * HP8  # 415: expert matmul cols per tile (80 valid per 83, rest junk)
WSCALE = 64.0  # host scale on expert weights before e4m3 quantization
INV_S = 1.0 / S
# 3x3 taps as (dy, dx), paired for DoubleRow; 9th tap pairs with zeros
TAP_PAIRS = [
    ((0, 0), (0, 1)),
    ((0, 2), (1, 0)),
    ((1, 1), (1, 2)),
    ((2, 0), (2, 1)),
    ((2, 2), (2, 2)),  # second half has zero weights; AP uses stride +1
]

_cache = {}


def _build_program(reps=1):
    nc = bacc.Bacc(
        "TRN2",
        target_bir_lowering=False,
        debug=False,
        enable_asserts=True,
        dynamic_dma_scratch_size=4096,
    )
    x_d = nc.dram_tensor("x", [BPC, 2, C, 2, S], FP8, kind="ExternalInput").ap()
    w1_d = nc.dram_tensor("w1", [C, 3, 2, 2 * C], FP8, kind="ExternalInput").ap()
    b1_d = nc.dram_tensor("b1", [2 * C, 1], F32, kind="ExternalInput").ap()
    wr_d = nc.dram_tensor("wrt", [C, E], F32, kind="ExternalInput").ap()
    br_d = nc.dram_tensor("br", [1, E], F32, kind="ExternalInput").ap()
    wexp_d = nc.dram_tensor("wexp", [E * C, 5, 2, C], FP8, kind="ExternalInput").ap()
    w2_d = nc.dram_tensor("w2", [3 * C, 2 * C], BF16, kind="ExternalInput").ap()
    b2_d = nc.dram_tensor("b2", [2 * C, 1], F32, kind="ExternalInput").ap()
    out_d = nc.dram_tensor("out", [BPC, 2 * C, S], BF16, kind="ExternalOutput").ap()

    with tile.TileContext(nc) as tc:
        _emit(nc, tc, x_d, w1_d, b1_d, wr_d, br_d, wexp_d, w2_d, b2_d, out_d, reps)
    nc.compile()
    return nc


def _emit(nc, tc, x_d, w1_d, b1_d, wr_d, br_d, wexp_d, w2_d, b2_d, out_d, reps=1):
    from contextlib import ExitStack

    ctx = ExitStack()
    with ctx:
        wp = ctx.enter_context(tc.tile_pool(name="weights", bufs=1))
        sp = ctx.enter_context(tc.tile_pool(name="stream", bufs=2))
        pp = ctx.enter_context(tc.tile_pool(name="psum", bufs=1, space="PSUM"))

        # --- static weights -------------------------------------------------
        # w1 leads the SP queue; b1 trails the first x half-chunk on Pool
        # (emitted from p1_group); late-needed weights (w2/b2/wrt/br) are
        # DMA'd on SP after item 1's first x chunk (see late_weights()).
        w1 = wp.tile([C, 3, 2, 2 * C], FP8, name="w1sb")
        nc.sync.dma_start(w1[:], w1_d)
        b1 = wp.tile([C, 2], F32, name="b1sb")

        def b1_dma():
            for mt in range(2):
                nc.gpsimd.dma_start(b1[:, mt : mt + 1], b1_d[mt * C : (mt + 1) * C, :])
        wrt = wp.tile([C, E], F32, name="wrtsb")
        br = wp.tile([1, E], F32, name="brsb")
        w2c = [wp.tile([C, 2 * C], BF16, name=f"w2c{j}") for j in range(3)]
        b2 = wp.tile([C, 2], F32, name="b2sb")

        def late_weights():
            for j in range(3):
                nc.sync.dma_start(w2c[j][:], w2_d[j * C : (j + 1) * C, :])
            for mt in range(2):
                nc.sync.dma_start(b2[:, mt : mt + 1], b2_d[mt * C : (mt + 1) * C, :])

        ones = wp.tile([1, C], F32, name="ones")
        nc.vector.memset(ones[:], 1.0)
        zrow = wp.tile([C, HP], BF16, name="zrow")
        nc.vector.memset(zrow[:], 0.0)
        # trigger the silu activation-table load at t~0 instead of right
        # before the first real activation (saves its 1.3us from the path)
        atl = wp.tile([1, 1], F32, name="atl")
        nc.scalar.activation(atl[:], ones[0:1, 0:1], AF.Silu)

        # all-4 expert weights staged in SBUF; routing then selects with a
        # DVE copy at a dynamic offset (no DMA latency / queue coupling)
        wall = wp.tile([C, E * 1280], FP8, name="wall")

        def prefetch_experts():
            nc.sync.dma_start(wrt[:], wr_d)
            nc.sync.dma_start(br[:], br_d)
            for e in range(E):
                nc.gpsimd.dma_start(
                    wall[:, e * 1280 : (e + 1) * 1280],
                    wexp_d[e * C : (e + 1) * C, :, :, :],
                )

        def p1_state(b):
            sa = sp.tile([C, S], BF16, tag="sa", bufs=2)
            mp = sp.tile([C, HP, HP], BF16, tag="mp", bufs=2)
            # mp8 rows: 0..4 all-ones (the bias pair of every expert
            # matmul reads a flat 415-col window here via a negative pair
            # stride; 64*bexp sits in partition 0 of the pair-5 lhs second
            # half). Rows 5..86 hold the zero-padded m map, 87 spare-zero.
            # Ones at the FRONT so an expert matmul's AP range only spans
            # rows already produced (at the end, every exp matmul would
            # wait on its item's final mp8 copy).
            mp8 = sp.tile([C, HP8 + 5, HP8], FP8, tag="mp8", bufs=2)
            parts = sp.tile([C, NG], BF16, tag="parts", bufs=2)
            nc.vector.tensor_copy(mp[:, 0:1, :], zrow[:, None, :])
            nc.vector.tensor_copy(mp[:, HP - 1 : HP, :], zrow[:, None, :])
            nc.vector.tensor_copy(mp[:, 1 : HP - 1, 0:1], zrow[:, 0 : HP - 2, None])
            nc.vector.tensor_copy(
                mp[:, 1 : HP - 1, HP - 1 : HP], zrow[:, 0 : HP - 2, None]
            )
            # zero the spare row/col of the fp8 copy (read as junk columns
            # by the full-row expert matmuls; must be finite)
            nc.gpsimd.memset(mp8[:, 0:5, :], 1.0)
            nc.gpsimd.memset(mp8[:, 5:, HP : HP + 1], 0.0)
            nc.gpsimd.memset(mp8[:, 5 + HP : 6 + HP, :], 0.0)
            return sa, mp, mp8, parts

        def part_reduce(mp, parts, g):
            # pooled partial sum on DVE (the Act accumulator costs 187ns of
            # Act time per use; the Act engine is the span bottleneck)
            base = mp[:, 1 + 10 * g, 0:1]
            flat = AP(base.tensor, base.offset, [base.ap[0], [1, 10 * HP]])
            # bf16 partials: 2x DVE mode halves the reduce; pooled noise
            # stays ~16x under the routing top-2 margin
            with nc.allow_low_precision("bf16 pooled partials, margin checked"):
                nc.vector.tensor_reduce(
                    parts[:, g : g + 1], flat, op=ALU.add, axis=mybir.AxisListType.X
                )

        def p1_chunk(b, chunk, xs):
            if xs[chunk] is not None:
                return xs[chunk]
            xg = sp.tile([C, 2, 2, 4 * N], FP8, tag="x", bufs=6, name=f"x{b}_{chunk}")
            if b == 0 and chunk == 0:
                # startup: b1 leads the Pool queue; the first tile's 400
                # cols arrive via small leading DMAs so the first act can
                # fire ~1us sooner (w1's Ah blob is also DMA'd first)
                nc.gpsimd.dma_start(xg[:, 0, :, 0:N], x_d[b, 0, :, :, 0:N])
                nc.sync.dma_start(xg[:, 1, :, 0:N], x_d[b, 1, :, :, 0:N])
                b1_dma()
                nc.gpsimd.dma_start(
                    xg[:, 0, :, N : 4 * N], x_d[b, 0, :, :, N : 4 * N]
                )
                nc.sync.dma_start(xg[:, 1, :, N : 4 * N], x_d[b, 1, :, :, N : 4 * N])
            else:
                # item 0's chunks split across the Pool/SP queues so the
                # startup isn't starved behind a single serial DMA queue
                for h in range(2):
                    eng = nc.gpsimd if (b == 0 and h == 0) else nc.sync
                    eng.dma_start(
                        xg[:, h, :, :],
                        x_d[b, h, :, :, chunk * 4 * N : (chunk + 1) * 4 * N],
                    )
            xs[chunk] = xg
            return xg

        def p1_half(b, g, st, xs, half, ptag=None):
            """One branch (half 0 = a/mt0, half 1 = m/mt1) of a cv1 group.

            cv1 runs as fp8 hi/lo DoubleRow: psum = 64*w1.T @ x via 3 DR
            matmuls Ah@xh + Al16@xh + Ah16@xl (w1 blobs [Ah, Ah16, Al16]).
            The halves are independent (separate psum tiles) so they can be
            scheduled into different p2 cycles as act filler work.
            """
            sa, mp, mp8, parts = st
            xg = p1_chunk(b, g // 2, xs)
            off = (g % 2) * GN
            mt = half
            # preamble a-halves use the (then idle) pse region; everything
            # emitted inside the p2 loop uses pso so the pse slot rotation
            # stays a pure exp stream (pse0/pse1 only ever wait exp acts)
            tag = ptag or ("pse" if half == 0 else "pso")
            ps = pp.tile([C, 2, 512], F32, tag=tag, bufs=2, name=f"p1{tag}_{b}_{g}")

            def xg_rhs(h, i):
                base = xg[:, h, 0, off + i * N]
                return AP(base.tensor, base.offset, [base.ap[0], [4 * N, 2], [1, N]])

            ms = slice(mt * C, (mt + 1) * C)

            def emit_act(i0, i1):
                if half == 0:
                    nc.scalar.activation(
                        sa[:, g * GN + i0 * N : g * GN + i1 * N],
                        ps[:, i0:i1, 0:N],
                        AF.Silu,
                        bias=b1[:, 0:1],
                        scale=1.0 / WSCALE,
                    )
                else:
                    # the last group's partial is routing-critical: take it
                    # from the Act accumulator (+187ns of Act time) so
                    # pooled doesn't wait ~900ns on a trailing DVE reduce
                    accum = (g == NG - 1 or (b == 1 and g <= 1)) and i1 == 2
                    with nc.allow_low_precision("bf16 pooled partials"):
                        nc.scalar.activation(
                            mp[:, 1 + 10 * g + 5 * i0 : 1 + 10 * g + 5 * i1, 1 : 1 + W],
                            ps[:, i0:i1, 0:N],
                            AF.Silu,
                            bias=b1[:, 1:2],
                            scale=1.0 / WSCALE,
                            accum_out=(parts[:, g : g + 1] if accum else None),
                        )

            first = b == 0 and g == 0
            for i in range(2):
                for s, (q, h) in enumerate([(0, 0), (2, 0), (1, 1)]):
                    nc.tensor.matmul(
                        ps[:, i, 0:N],
                        w1[:, q, :, ms],
                        xg_rhs(h, i),
                        start=(s == 0),
                        stop=(s == 2),
                        perf_mode=DR,
                    )
            if first:
                emit_act(0, 1)
                emit_act(1, 2)
            else:
                emit_act(0, 2)
            if half == 1:
                if g != NG - 1 and not (b == 1 and g <= 1):
                    part_reduce(mp, parts, g)
                # fp8 copy of the padded m rows for the expert convs
                # (gpsimd); extend to the border rows at the ends
                r0 = 0 if g == 0 else 1 + 10 * g
                r1 = HP if g == NG - 1 else 11 + 10 * g
                nc.gpsimd.tensor_copy(mp8[:, 5 + r0 : 5 + r1, 0:HP], mp[:, r0:r1, :])

        def routing(b, st):
            with tc.high_priority():
                return _routing(b, st)

        def _routing(b, st):
            sa, mp, mp8, parts = st
            pooled = sp.tile([C, 1], F32, tag="pooled", bufs=2)
            nc.vector.reduce_sum(pooled[:], parts[:], axis=mybir.AxisListType.X)
            # logits in [1, E] layout directly (pooled as stationary operand)
            # so no partition-transpose DMA is needed before the top-2 scan
            ps_l = pp.tile([1, E], F32, tag="pse", bufs=2)
            nc.tensor.matmul(ps_l[:], pooled[:], wrt[:], start=True, stop=True)
            row = sp.tile([1, 8], F32, tag="row", bufs=2)
            nc.vector.memset(row[:], -1e30)
            nc.vector.tensor_scalar(row[0:1, 0:E], ps_l[:], INV_S, None, op0=ALU.mult)
            nc.vector.tensor_tensor(row[0:1, 0:E], row[0:1, 0:E], br[:], op=ALU.add)
            vals = sp.tile([1, 8], F32, tag="vals", bufs=2)
            nc.vector.max(vals[:], row[:])
            uidx = sp.tile([1, 8], mybir.dt.uint32, tag="uidx", bufs=2)
            nc.vector.max_index(uidx[:], vals[:], row[:])
            # gates: g0 = sigmoid(l0 - l1) = silu(d)/d, g1 = 1 - g0
            scr = sp.tile([1, 4], F32, tag="scr", bufs=2)
            nc.vector.tensor_tensor(
                scr[:, 0:1], vals[:, 0:1], vals[:, 1:2], op=ALU.subtract
            )
            nc.vector.reciprocal(scr[:, 1:2], scr[:, 0:1])
            nc.scalar.activation(scr[:, 2:3], scr[:, 0:1], AF.Silu)
            g = sp.tile([1, 2], F32, tag="g", bufs=2)
            nc.vector.tensor_tensor(g[:, 0:1], scr[:, 2:3], scr[:, 1:2], op=ALU.mult)
            nc.vector.tensor_scalar(
                g[:, 1:2], g[:, 0:1], -1.0, 1.0, op0=ALU.mult, op1=ALU.add
            )
            wks = []
            for k in range(2):
                iv = nc.values_load(
                    uidx[0:1, k : k + 1],
                    min_val=0,
                    max_val=E - 1,
                    skip_runtime_bounds_check=True,
                )
                wk = sp.tile([C, 1280], FP8, tag=f"expw{k}", bufs=2, name=f"expw{k}")
                nc.vector.tensor_copy(wk[:], wall[:, ds(iv * 1280, 1280)])
                wks.append(wk)

            g_bc = sp.tile([C, 2], F32, tag="gbc", bufs=2)

            def emit_gates():
                # the gates-broadcast matmul is emitted lazily (after the
                # first exp group's matmuls): emitted any earlier it sits
                # in the in-order PE queue waiting on the DVE gates chain
                # and barriers every matmul behind it for ~1.4us
                ps_g = pp.tile([C, 2], F32, tag="pso", bufs=2)
                nc.tensor.matmul(ps_g[:], ones[:], g[:], start=True, stop=True)
                nc.vector.tensor_copy(g_bc[:], ps_g[:])

            return [wks, g_bc, emit_gates]

        def pair_lhs(wk, p):
            """[C, 2, C] fp8 lhsT view of the flat gathered expert blob."""
            base = wk[:, p * 2 * C : p * 2 * C + 1]
            return AP(base.tensor, base.offset, [base.ap[0], [C, 2], [1, C]])

        def pair_rhs(mp8, t, pair):
            """[C, 2, NV] fp8 AP: two flat full-row windows of the m copy.

            The last pair's second half points at the ones rows (83..87) so
            its lhs can carry the expert bias (64*bexp at partition 0)."""
            (y0, x0), (y1, x1) = pair
            r0 = t * R
            base = mp8[:, 5 + r0 + y0, x0 : x0 + 1]
            if pair[0] == pair[1]:
                # bias pair: second half reads the ones rows at the front
                d = -((5 + r0 + y0) * HP8 + x0)
            else:
                d = (y1 - y0) * HP8 + (x1 - x0)
            return AP(base.tensor, base.offset, [base.ap[0], [d, 2], [1, NV]])

        def valid_cols(ps, bank0, nb=2):
            """[C, nb, R, W] f32 AP over the valid columns of PSUM banks."""
            base = ps[:, bank0, 0:1]
            return AP(
                base.tensor, base.offset, [base.ap[0], [512, nb], [HP8, R], [1, W]]
            )

        def exp_group(b, g, st, rt):
            sa, mp, mp8, parts = st
            wks, g_bc = rt[0], rt[1]
            pse = [
                pp.tile([C, 2, 512], F32, tag="pse", bufs=2, name=f"pse{k}_{b}_{g}")
                for k in range(2)
            ]
            sg = sp.tile([C, 2, GN], BF16, tag="sg", bufs=4, name=f"sg{b}_{g}")
            for k in range(2):
                for i in range(2):
                    t = 2 * g + i
                    for p in range(5):
                        nc.tensor.matmul(
                            pse[k][:, i, 0:NV],
                            pair_lhs(wks[k], p),
                            pair_rhs(mp8, t, TAP_PAIRS[p]),
                            start=(p == 0),
                            stop=(p == 4),
                            perf_mode=DR,
                        )
                # bias is pre-added in PSUM by the pair-5 ones trick, so
                # both acts are bias-free with a shared scale
                nc.scalar.activation(
                    sg[:, k, :],
                    valid_cols(pse[k], 0),
                    AF.Silu,
                    scale=1.0 / WSCALE,
                )
            if rt[2] is not None:
                rt[2]()
                rt[2] = None
            moe = sp.tile([C, GN], BF16, tag="moe", bufs=4, name=f"moe{b}_{g}")
            nc.vector.tensor_scalar_mul(moe[:], sg[:, 0, :], g_bc[:, 0:1])
            nc.vector.scalar_tensor_tensor(
                moe[:], sg[:, 1, :], g_bc[:, 1:2], moe[:], op0=ALU.mult, op1=ALU.add
            )
            return moe

        def cv2_group(b, g, st, moe, last=False):
            sa, mp, mp8, parts = st
            pso = [
                pp.tile([C, 2, 512], F32, tag="pso", bufs=2, name=f"pso{mt}_{b}_{g}")
                for mt in range(2)
            ]
            for mt in range(2):
                ms = slice(mt * C, (mt + 1) * C)
                for i in range(2):
                    t = 2 * g + i
                    dst = pso[mt][:, i, 0:N]
                    nc.tensor.matmul(
                        dst, w2c[0][:, ms], sa[:, t * N : (t + 1) * N],
                        start=True, stop=False,
                    )
                    nc.tensor.matmul(
                        dst, w2c[1][:, ms],
                        mp[:, 1 + t * R : 1 + (t + 1) * R, 1 : 1 + W],
                        start=False, stop=False,
                    )
                    nc.tensor.matmul(
                        dst, w2c[2][:, ms], moe[:, i * N : (i + 1) * N],
                        start=False, stop=True,
                    )
            # the final group splits its acts/stores into 400-col pieces so
            # the last DMA overlaps the last activation instead of trailing
            # a full 800-col one
            nh = 1
            for mt in range(2):
                ms = slice(mt * C, (mt + 1) * C)
                for h in range(2 // nh * 0 + nh):
                    w0 = g * GN + h * (GN // nh)
                    ot = sp.tile(
                        [C, GN // nh], BF16, tag=f"ot{mt}", bufs=4, name=f"ot{mt}_{g}_{h}"
                    )
                    nc.scalar.activation(
                        ot[:],
                        pso[mt][:, h * (2 // nh) : (h + 1) * (2 // nh), 0:N],
                        AF.Silu,
                        bias=b2[:, mt : mt + 1],
                    )
                    # item 0's outs all go on SP so Pool's queue stays short
                    # for the expert weight gathers (in-order queues; a
                    # gather stuck behind out DMAs stalls the expert
                    # matmuls). By item 1's p2 phase Pool is idle, so its
                    # outs split across both queues (and the final group goes
                    # entirely to Pool, which is empty by then).
                    if last:
                        eng = nc.gpsimd if (mt + h) % 2 == 0 else nc.sync
                    elif b == 1 and mt == 1:
                        eng = nc.gpsimd
                    else:
                        eng = nc.sync
                    if last and mt == 1:
                        # the very last store gates the drain: split it
                        # across both queues so the transfers run in half
                        nc.gpsimd.dma_start(out_d[b, ms, w0 : w0 + N], ot[:, 0:N])
                        nc.sync.dma_start(
                            out_d[b, ms, w0 + N : w0 + GN], ot[:, N:GN]
                        )
                    else:
                        eng.dma_start(out_d[b, ms, w0 : w0 + GN // nh], ot[:])

        for _rep in range(reps):
            xs = [[None] * 4, [None] * 4]
            st = [None, None]
            # first x chunk's DMAs lead every queue (before p1_state's
            # memsets, which would delay the Pool-queue x piece by ~0.5us)
            p1_chunk(0, 0, xs[0])
            st[0] = p1_state(0)
            # preamble p1: all of item 0's m halves (routing(0) + mp8 need
            # them) and its first two a halves (cv2(0,0/1) at j=1,2); the
            # remaining a halves defer into p2 as act filler work
            for g in range(2):
                p1_half(0, g, st[0], xs[0], 0)
                p1_half(0, g, st[0], xs[0], 1)
            for g in range(2, NG):
                p1_half(0, g, st[0], xs[0], 1)
            prefetch_experts()
            st[1] = p1_state(1)
            p1_half(1, 0, st[1], xs[1], 1)
            late_weights()
            p1_half(1, 1, st[1], xs[1], 1)
            p1_half(1, 2, st[1], xs[1], 1)
            # routing(0) goes after item 1's preamble halves: their m-acts
            # are the only act work that can cover the pooled->top2->gather
            # latency, and emitting their matmuls first keeps them off the
            # logits matmul's in-order PE barrier (they use Act-accum
            # partials, so no DVE reduces sit ahead of the routing chain)
            rt = [routing(0, st[0]), None]
            # p2: one continuous software pipeline over both items' groups;
            # cv2 lags exp by one group so the exp->act->moe chain of a group
            # resolves while the PE runs cv2 of the previous one. The
            # deferred p1 halves spread across the cycles as filler work:
            # the late (item-1) cycles are PE-sequence-bound (exp+cv2 mms =
            # ~3.7us > 3.4us of act work), so the fillers' acts are what
            # keeps the Act engine saturated there.
            pairs = [(b, g) for b in range(BPC) for g in range(NG)]
            fill = [[] for _ in range(len(pairs) + 1)]
            for k, g in enumerate(range(3, NG)):
                fill[k].append((1, g, 1))  # m(1,g) at j=0..4 (routing(1))
            for k, g in enumerate(range(2, NG)):
                fill[k].append((0, g, 0))  # a(0,g) at j=0..5 (cv2(0,g) at g+1)
            for k, g in enumerate(range(0, NG)):
                fill[6 + k].append((1, g, 0))  # a(1,g) at j=6..13
            moes = [None] * len(pairs)
            for j in range(len(pairs) + 1):
                if j < len(pairs):
                    b, g = pairs[j]
                    moes[j] = exp_group(b, g, st[b], rt[b])
                if j >= 1:
                    b, g = pairs[j - 1]
                    cv2_group(b, g, st[b], moes[j - 1], last=(j == len(pairs)))
                # fillers go last: their psum slot then waits an act that is
                # AHEAD of theirs in the in-order Act queue (fills-first
                # would head-of-line deadlock via the pso rotation)
                for b, g, half in fill[j]:
                    p1_half(b, g, st[b], xs[b], half, ptag="pso")
                if j == 6:
                    rt[1] = routing(1, st[1])


def _prep_inputs(x, W_cv1, b_cv1, W_r, b_r, W_exp, b_exp, W_cv2, b_cv2):
    """Host-side packing shared by kernel() and the test harness."""
    BF = ml_dtypes.bfloat16
    E4 = ml_dtypes.float8_e4m3fn
    x = np.asarray(x, np.float32)
    # x: [B, C1, S] -> hi/lo e4m3 split packed [B, 2(hl), 128, 2(k), S]
    xk = x.reshape(B, 2, C, S)  # [B, k, c, S]
    xh = xk.astype(E4)
    xl = (16.0 * (xk - xh.astype(np.float32))).astype(E4)
    xp = np.ascontiguousarray(
        np.stack([xh, xl], axis=1).transpose(0, 1, 3, 2, 4)
    )  # [B, hl, c, k, S]
    # w1: [256out, 256in] -> 3 fp8 blobs [C, 3, k, out]:
    #   A = 64*w1^T; blobs = [Ah, Ah/16, e4(16*(A-Ah))/16]
    w1 = np.asarray(W_cv1, np.float32)[:, :, 0, 0]  # [2C out, C1 in]
    A = (WSCALE * w1.T).astype(np.float32)  # [in 256, out 256]
    Ah = A.astype(E4)
    Al = (16.0 * (A - Ah.astype(np.float32))).astype(E4)
    Ah16 = (Ah.astype(np.float32) / 16.0).astype(E4)
    Al16 = (Al.astype(np.float32) / 16.0).astype(E4)
    blobs = [
        b.reshape(2, C, 2 * C).transpose(1, 0, 2) for b in (Ah, Ah16, Al16)
    ]
    w1p = np.ascontiguousarray(np.stack(blobs, axis=1))  # [C, 3, k, 2C]
    # expert weights: [E, out, in, ky, kx] scaled, e4m3, packed [E*C, 5, 2, C]
    we = np.asarray(W_exp, np.float32) * WSCALE
    weq = we.astype(E4)
    wexp = np.zeros((E, C, 5, 2, C), E4)
    for p, (t0, t1) in enumerate(TAP_PAIRS):
        wexp[:, :, p, 0, :] = weq[:, :, :, t0[0], t0[1]].transpose(0, 2, 1)
        if p < 4:
            wexp[:, :, p, 1, :] = weq[:, :, :, t1[0], t1[1]].transpose(0, 2, 1)
    # pair-5 second half: partition 0 carries 64*bexp (its rhs window reads
    # the all-ones rows of mp8), so the conv bias lands in PSUM for free
    wexp[:, 0, 4, 1, :] = (np.asarray(b_exp, np.float32) * WSCALE).astype(E4)
    wexp = np.ascontiguousarray(wexp.reshape(E * C, 5, 2, C))
    w2 = np.asarray(W_cv2, np.float32)[:, :, 0, 0]  # [256 out, 384 in]
    w2p = np.ascontiguousarray(w2.T.astype(BF))  # [384, 256]
    shared = {
        "w1": w1p,
        "b1": np.asarray(b_cv1, np.float32).reshape(-1, 1),
        "wrt": np.ascontiguousarray(np.asarray(W_r, np.float32).T),
        "br": np.asarray(b_r, np.float32).reshape(1, E),
        "wexp": wexp,
        "w2": w2p,
        "b2": np.asarray(b_cv2, np.float32).reshape(-1, 1),
    }
    return [
        {**shared, "x": np.ascontiguousarray(xp[i * BPC : (i + 1) * BPC])}
        for i in range(N_CORES)
    ]


def kernel(x, W_cv1, b_cv1, W_r, b_r, W_exp, b_exp, W_cv2, b_cv2):
    if "nc" not in _cache:
        _cache["nc"] = _build_program()
    nc = _cache["nc"]
    in_maps = _prep_inputs(x, W_cv1, b_cv1, W_r, b_r, W_exp, b_exp, W_cv2, b_cv2)
    res = run_bass_kernel_spmd(nc, in_maps, core_ids=list(range(N_CORES)))
    _cache["last_results"] = res
    out = np.concatenate(
        [np.asarray(res.results[i]["out"], np.float32) for i in range(N_CORES)], axis=0
    )
    return out.reshape(B, 2 * C, H, W)

